# revision 6
# baseline (speedup 1.0000x reference)
"""Bidirectional LSTM layer (T=256, B=64, I=H=512) on 8 TRN2 NeuronCores.

Sharding (MODE="local"): core = dir(2) x batch-shard(4). Each core runs the
full recurrence for its direction on a 16-sample batch slice, bf16 matmuls
with f32 PSUM accumulation and f32 cell state. The backward direction is
handled by feeding time-reversed x to cores 4-7 and re-reversing on the host,
so all 8 cores run one identical SPMD graph.

Sharding (MODE="tp4"): core = dir(2) x gate-tensor-parallel(4). Each core owns
a 128-dim H-slice of all four gates (full batch), exchanging h chunks with its
3 group peers each step via remote_dma_broadcast (XOR-relative addressing).

Per-step structure (both modes):
  PSUM gates[rt] = I @ xg_t[rt] + sum_k WhT[k, rt] @ hT[k]   (one accum group)
  ACT: sigmoid(i,f,o), tanh(g); DVE: c = f*c + i*g; ACT tanh(c); DVE h = o*tanh(c)
xg = x @ Wi^T + bi + bh is precomputed on-device as one large bf16 GEMM and
kept in SBUF (bf16) for the whole recurrence.
"""

import sys

for p in ("/opt/trn_rl_repo",):
    if p not in sys.path:
        sys.path.insert(0, p)

import numpy as np
import ml_dtypes

import concourse.bass as bass
import concourse.tile as tile
import concourse.mybir as mybir
from concourse import bacc, bass_utils

BF16 = ml_dtypes.bfloat16
F32 = np.float32

T, B, I, H = 256, 64, 512, 512
GATE_ORDER = [0, 1, 3, 2]  # pytorch i,f,g,o -> our row-tile order i,f,o,g

MODE = "local"  # "local" (no cross-core comm) or "tp4" (gate TP + remote_dma)

TRACE = False  # set by test harness to capture neuron-profile timing
LAST_RESULT = None  # BassKernelResults of the most recent run

_GRAPH_CACHE = {}


def _cfg(mode):
    if mode == "local":
        # Bs batch per core, RT gate row-tiles per core (16 = all 2048 rows)
        return dict(Bs=16, RT=16, use_remote=False)
    elif mode == "tp4":
        return dict(Bs=64, RT=4, use_remote=True)
    raise ValueError(mode)


def build_graph(mode, Tn=T):
    cfg = _cfg(mode)
    Bs, RT, use_remote = cfg["Bs"], cfg["RT"], cfg["use_remote"]
    RT4 = RT // 4
    TBs = Tn * Bs
    NCH = 8  # xg chunks along time
    CH = TBs // NCH  # columns per xg chunk
    NPW = min(512, CH)  # N-pass width
    NB = CH // NPW  # N passes per chunk
    dt = mybir.dt

    nc = bacc.Bacc("TRN2", target_bir_lowering=False, debug=False, num_devices=8)

    xT_d = nc.dram_tensor("xT", [4, 128, TBs], dt.bfloat16, kind="ExternalInput").ap()
    wiT_d = nc.dram_tensor("wiT", [4, 128, RT * 128], dt.bfloat16, kind="ExternalInput").ap()
    whT_d = nc.dram_tensor("whT", [4, 128, RT * 128], dt.bfloat16, kind="ExternalInput").ap()
    bias_d = nc.dram_tensor("bias", [128, RT], dt.float32, kind="ExternalInput").ap()
    h0T_d = nc.dram_tensor("h0T", [128, 4, Bs], dt.bfloat16, kind="ExternalInput").ap()
    c0T_d = nc.dram_tensor("c0T", [128, RT4, Bs], dt.float32, kind="ExternalInput").ap()
    id_d = nc.dram_tensor("ident", [128, 128], dt.bfloat16, kind="ExternalInput").ap()

    ysT_d = nc.dram_tensor("ysT", [Tn, 128, RT4 * Bs], dt.bfloat16, kind="ExternalOutput").ap()
    hfin_d = nc.dram_tensor("hfin", [128, RT4 * Bs], dt.float32, kind="ExternalOutput").ap()
    cfin_d = nc.dram_tensor("cfin", [128, RT4 * Bs], dt.float32, kind="ExternalOutput").ap()

    if use_remote:
        h_sem = nc.alloc_semaphore("h_sem")
        send_sem = nc.alloc_semaphore("send_sem")

    AF = mybir.ActivationFunctionType

    with tile.TileContext(nc) as tc:
        with (
            tc.tile_pool(name="persist", bufs=1) as persist,
            tc.tile_pool(name="xc", bufs=3) as xcp,
            tc.tile_pool(name="gpsum", bufs=3, space="PSUM") as gpsum,
            tc.tile_pool(name="spsum", bufs=2, space="PSUM") as spsum,
            tc.tile_pool(name="ew", bufs=2) as ew,
        ):
            wiT = persist.tile([128, 4, RT * 128], dt.bfloat16, tag="wiT")
            whT = persist.tile([128, 4, RT * 128], dt.bfloat16, tag="whT")
            bias = persist.tile([128, RT], dt.float32, tag="bias")
            ident = persist.tile([128, 128], dt.bfloat16, tag="ident")
            hT = persist.tile([128, 2, 4, Bs], dt.bfloat16, tag="hT")
            c_sb = persist.tile([128, RT4, Bs], dt.float32, tag="c_sb")
            xg = [
                persist.tile([128, RT, CH], dt.bfloat16, tag=f"xg{ch}", name=f"xg{ch}")
                for ch in range(NCH)
            ]

            for k in range(4):
                nc.sync.dma_start(wiT[:, k, :], wiT_d[k])
                nc.sync.dma_start(whT[:, k, :], whT_d[k])
            nc.sync.dma_start(bias[:], bias_d[:])
            nc.sync.dma_start(ident[:], id_d[:])
            nc.sync.dma_start(hT[:, 0, :, :], h0T_d[:])
            nc.sync.dma_start(c_sb[:], c0T_d[:])

            # ---- Phase 1: xg = x @ Wi^T + bias, bf16, resident in SBUF ----
            for ch in range(NCH):
                for nb in range(NB):
                    off = ch * CH + nb * NPW
                    xc = xcp.tile([128, 4, NPW], dt.bfloat16, tag="xc")
                    for k in range(4):
                        nc.sync.dma_start(xc[:, k, :], xT_d[k, :, off : off + NPW])
                    for rt in range(RT):
                        ps = gpsum.tile([128, NPW], dt.float32, tag="gps")
                        for k in range(4):
                            nc.tensor.matmul(
                                ps[:],
                                wiT[:, k, rt * 128 : (rt + 1) * 128],
                                xc[:, k, :],
                                start=(k == 0),
                                stop=(k == 3),
                            )
                        dst = xg[ch][:, rt, nb * NPW : (nb + 1) * NPW]
                        if rt % 2 == 0:
                            nc.scalar.activation(
                                dst, ps[:], AF.Identity, bias=bias[:, rt : rt + 1]
                            )
                        else:
                            nc.vector.tensor_scalar_add(dst, ps[:], bias[:, rt : rt + 1])

            # ---- Phase 2: recurrence ----
            TCH = Tn // NCH  # steps per xg chunk
            for t in range(Tn):
                cur, nxt = t % 2, (t + 1) % 2
                ch, tloc = t // TCH, t % TCH
                xg_t = xg[ch]
                ps = spsum.tile([128, RT, Bs], dt.float32, tag="sps")

                if use_remote and t >= 1:
                    with tc.tile_critical():
                        nc.tensor.wait_ge(h_sem, 6 * t)
                        for rt in range(RT):
                            first = rt == 0
                            nc.tensor.matmul(
                                ps[:, rt, :],
                                ident[:],
                                xg_t[:, rt, tloc * Bs : (tloc + 1) * Bs],
                                start=first,
                                stop=False,
                                skip_group_check=True,
                            )
                            for k in range(4):
                                nc.tensor.matmul(
                                    ps[:, rt, :],
                                    whT[:, k, rt * 128 : (rt + 1) * 128],
                                    hT[:, cur, k, :],
                                    start=False,
                                    stop=(rt == RT - 1 and k == 3),
                                    skip_group_check=True,
                                )
                else:
                    for rt in range(RT):
                        first = rt == 0
                        nc.tensor.matmul(
                            ps[:, rt, :],
                            ident[:],
                            xg_t[:, rt, tloc * Bs : (tloc + 1) * Bs],
                            start=first,
                            stop=False,
                            skip_group_check=True,
                        )
                        for k in range(4):
                            nc.tensor.matmul(
                                ps[:, rt, :],
                                whT[:, k, rt * 128 : (rt + 1) * 128],
                                hT[:, cur, k, :],
                                start=False,
                                stop=(rt == RT - 1 and k == 3),
                                skip_group_check=True,
                            )

                # elementwise: row-tiles [0:RT4]=i [RT4:2RT4]=f [2RT4:3RT4]=o [3RT4:RT]=g
                sig = ew.tile([128, 3 * RT4, Bs], dt.float32, tag="sig")
                tng = ew.tile([128, RT4, Bs], dt.float32, tag="tng")
                nc.scalar.activation(sig[:], ps[:, 0 : 3 * RT4, :], AF.Sigmoid)
                nc.scalar.activation(tng[:], ps[:, 3 * RT4 : RT, :], AF.Tanh)
                ig = ew.tile([128, RT4, Bs], dt.float32, tag="ig")
                fc = ew.tile([128, RT4, Bs], dt.float32, tag="fc")
                nc.vector.tensor_mul(ig[:], sig[:, 0:RT4, :], tng[:])
                nc.vector.tensor_mul(fc[:], sig[:, RT4 : 2 * RT4, :], c_sb[:])
                nc.vector.tensor_add(c_sb[:], ig[:], fc[:])
                tcn = ew.tile([128, RT4, Bs], dt.float32, tag="tc")
                nc.scalar.activation(tcn[:], c_sb[:], AF.Tanh)
                if use_remote:
                    hdst = hT[:, nxt, 0:RT4, :]
                else:
                    hdst = hT[:, nxt, :, :]
                nc.vector.tensor_mul(hdst, sig[:, 2 * RT4 : 3 * RT4, :], tcn[:])

                if use_remote:
                    with tc.tile_critical():
                        for d in (1, 2, 3):
                            nc.gpsimd.remote_dma_broadcast(
                                hT[:, nxt, d, :],
                                hT[:, nxt, 0, :],
                                remote_sem=h_sem,
                                local_sem=send_sem,
                                rdests=[(0, d)] + [None] * 7,
                            )
                        nc.gpsimd.trigger_dma(count=None)

                nc.sync.dma_start(ysT_d[t], hT[:, nxt, 0:RT4, :])

                if t == Tn - 1:
                    hf = ew.tile([128, RT4, Bs], dt.float32, tag="hf")
                    nc.vector.tensor_mul(hf[:], sig[:, 2 * RT4 : 3 * RT4, :], tcn[:])
                    nc.sync.dma_start(hfin_d[:], hf[:])
                    nc.sync.dma_start(cfin_d[:], c_sb[:])

    nc.compile()
    return nc


def _rows_for(mode, rank):
    """Global gate-row indices (into 4H) for this core, in row-tile order."""
    cfg = _cfg(mode)
    RT = cfg["RT"]
    if mode == "local":
        return np.concatenate([512 * q + np.arange(512) for q in GATE_ORDER])
    else:
        return np.concatenate(
            [512 * q + 128 * rank + np.arange(128) for q in GATE_ORDER]
        )


def _prep_core(mode, c, x, h0, c0, Wi, Wh, bi, bh):
    cfg = _cfg(mode)
    Bs, RT = cfg["Bs"], cfg["RT"]
    RT4 = RT // 4
    d, rank = c // 4, c % 4
    rows = _rows_for(mode, rank)

    if mode == "local":
        bsl = slice(rank * Bs, (rank + 1) * Bs)
        hsl = np.arange(H)
        slot_slices = np.arange(4)  # hT slot j <- H-slice j
    else:
        bsl = slice(0, B)
        hsl = 128 * rank + np.arange(128)
        slot_slices = np.array([rank ^ j for j in range(4)])

    xx = x[::-1] if d == 1 else x
    xx = xx[:, bsl, :]  # [T, Bs, I]
    Tn = xx.shape[0]
    xT = np.ascontiguousarray(
        xx.astype(BF16).transpose(2, 0, 1).reshape(4, 128, Tn * Bs)
    )

    wi = Wi[rows].astype(BF16)  # [RT*128, I]
    wiT = np.ascontiguousarray(wi.T.reshape(4, 128, RT * 128))
    wh = Wh[rows].astype(BF16).T  # [H, RT*128]
    whT = np.stack(
        [wh[128 * s : 128 * s + 128] for s in slot_slices], axis=0
    )  # [4, 128, RT*128]
    bias = (bi + bh)[rows].astype(F32).reshape(RT, 128).T.copy()  # [128, RT]

    h0T = np.stack(
        [h0[bsl, 128 * s : 128 * s + 128].T.astype(BF16) for s in slot_slices], axis=1
    )  # [128, 4, Bs]
    if mode == "local":
        c0T = np.ascontiguousarray(c0[bsl].T.astype(F32).reshape(RT4, 128, Bs).transpose(1, 0, 2))
    else:
        c0T = c0[bsl, hsl.min() : hsl.min() + 128].T.astype(F32).reshape(128, 1, Bs)

    return {
        "xT": xT,
        "wiT": np.ascontiguousarray(wiT),
        "whT": np.ascontiguousarray(whT),
        "bias": np.ascontiguousarray(bias),
        "h0T": np.ascontiguousarray(h0T),
        "c0T": np.ascontiguousarray(c0T),
        "ident": np.eye(128, dtype=BF16),
    }


def kernel(x, h0_f, c0_f, h0_b, c0_b, Wi_f, Wh_f, bi_f, bh_f, Wi_b, Wh_b, bi_b, bh_b):
    mode = MODE
    cfg = _cfg(mode)
    Bs, RT = cfg["Bs"], cfg["RT"]
    RT4 = RT // 4
    x = np.asarray(x, dtype=F32)
    Tn = x.shape[0]

    if mode not in _GRAPH_CACHE:
        _GRAPH_CACHE[mode] = build_graph(mode, Tn)
    nc = _GRAPH_CACHE[mode]

    per_dir = [
        (h0_f, c0_f, Wi_f, Wh_f, bi_f, bh_f),
        (h0_b, c0_b, Wi_b, Wh_b, bi_b, bh_b),
    ]
    in_maps = []
    for c in range(8):
        h0, c0, Wi, Wh, bi, bh = [np.asarray(a, dtype=F32) for a in per_dir[c // 4]]
        in_maps.append(_prep_core(mode, c, x, h0, c0, Wi, Wh, bi, bh))

    res = bass_utils.run_bass_kernel_spmd(
        nc, in_maps, core_ids=list(range(8)), trace=TRACE
    )
    global LAST_RESULT
    LAST_RESULT = res

    out = np.zeros((Tn, B, 2 * H), dtype=F32)
    hf = np.zeros((B, H), dtype=F32)
    cf = np.zeros((B, H), dtype=F32)
    hb = np.zeros((B, H), dtype=F32)
    cb = np.zeros((B, H), dtype=F32)
    for c in range(8):
        d, rank = c // 4, c % 4
        r = res.results[c]
        ys = np.asarray(r["ysT"]).astype(F32).reshape(Tn, 128, RT4, Bs)
        hfin = np.asarray(r["hfin"]).astype(F32).reshape(128, RT4, Bs)
        cfin = np.asarray(r["cfin"]).astype(F32).reshape(128, RT4, Bs)
        ys = ys.transpose(0, 3, 2, 1).reshape(Tn, Bs, RT4 * 128)  # [T, Bs, dims]
        hfin = hfin.transpose(2, 1, 0).reshape(Bs, RT4 * 128)
        cfin = cfin.transpose(2, 1, 0).reshape(Bs, RT4 * 128)
        if d == 1:
            ys = ys[::-1]
        if mode == "local":
            bsl = slice(rank * Bs, (rank + 1) * Bs)
            dsl = slice(0, H)
        else:
            bsl = slice(0, B)
            dsl = slice(rank * 128, rank * 128 + 128)
        out[:, bsl, (d * H + dsl.start) : (d * H + dsl.stop)] = ys
        (hf if d == 0 else hb)[bsl, dsl] = hfin
        (cf if d == 0 else cb)[bsl, dsl] = cfin

    return out, hf, cf, hb, cb


if __name__ == "__main__":
    rng = np.random.default_rng(0)
    ins = {
        "x": rng.standard_normal((T, B, I), dtype=F32),
        "h0_f": np.zeros((B, H), F32),
        "c0_f": np.zeros((B, H), F32),
        "h0_b": np.zeros((B, H), F32),
        "c0_b": np.zeros((B, H), F32),
    }
    for dd in ("f", "b"):
        ins[f"Wi_{dd}"] = (rng.standard_normal((4 * H, I), dtype=F32) / np.sqrt(I)).astype(F32)
        ins[f"Wh_{dd}"] = (rng.standard_normal((4 * H, H), dtype=F32) / np.sqrt(H)).astype(F32)
        ins[f"bi_{dd}"] = rng.standard_normal(4 * H, dtype=F32) / np.sqrt(H)
        ins[f"bh_{dd}"] = rng.standard_normal(4 * H, dtype=F32) / np.sqrt(H)
    outs = kernel(**ins)
    print([o.shape for o in outs])


# revision 16
# speedup vs baseline: 5429.9851x; 5429.9851x over previous
"""Bidirectional LSTM layer (T=256, B=64, I=H=512) on 8 TRN2 NeuronCores.

Sharding (MODE="local"): core = dir(2) x batch-shard(4). Each core runs the
full recurrence for its direction on a 16-sample batch slice, bf16 matmuls
with f32 PSUM accumulation and f32 cell state. The backward direction is
handled by feeding time-reversed x to cores 4-7 and re-reversing on the host,
so all 8 cores run one identical SPMD graph.

Sharding (MODE="tp4"): core = dir(2) x gate-tensor-parallel(4). Each core owns
a 128-dim H-slice of all four gates (full batch), exchanging h chunks with its
3 group peers each step via remote_dma_broadcast (XOR-relative addressing).

Per-step structure (both modes):
  PSUM gates[rt] = I @ xg_t[rt] + sum_k WhT[k, rt] @ hT[k]   (one accum group)
  ACT: sigmoid(i,f,o), tanh(g); DVE: c = f*c + i*g; ACT tanh(c); DVE h = o*tanh(c)
xg = x @ Wi^T + bi + bh is precomputed on-device as one large bf16 GEMM and
kept in SBUF (bf16) for the whole recurrence.
"""

import sys

for p in ("/opt/trn_rl_repo",):
    if p not in sys.path:
        sys.path.insert(0, p)

import numpy as np
import ml_dtypes

import concourse.bass as bass
import concourse.tile as tile
import concourse.mybir as mybir
from concourse import bacc, bass_utils

BF16 = ml_dtypes.bfloat16
F32 = np.float32

T, B, I, H = 256, 64, 512, 512
GATE_ORDER = [0, 1, 3, 2]  # pytorch i,f,g,o -> our row-tile order i,f,o,g

MODE = "local"  # "local" (no cross-core comm) or "tp4" (gate TP + remote_dma)

TRACE = False  # set by test harness to capture neuron-profile timing
LAST_RESULT = None  # BassKernelResults of the most recent run

_GRAPH_CACHE = {}


def _cfg(mode):
    if mode == "local":
        # Bs batch per core, RT gate row-tiles per core (16 = all 2048 rows)
        return dict(Bs=16, RT=16, use_remote=False)
    elif mode == "tp4":
        return dict(Bs=64, RT=4, use_remote=True)
    raise ValueError(mode)


def build_graph(mode, Tn=T, reps=1):
    # reps>1 re-runs the recurrence compute without I/O (timing calibration
    # only -- outputs stay those of rep 0).
    cfg = _cfg(mode)
    Bs, RT, use_remote = cfg["Bs"], cfg["RT"], cfg["use_remote"]
    RT4 = RT // 4
    TBs = Tn * Bs
    NCH = 8  # xg chunks along time
    CH = TBs // NCH  # columns per xg chunk
    NPW = min(512, CH)  # N-pass width
    NB = CH // NPW  # N passes per chunk
    dt = mybir.dt

    nc = bacc.Bacc("TRN2", target_bir_lowering=False, debug=False, num_devices=8)

    xT_d = nc.dram_tensor("xT", [4, 128, TBs], dt.bfloat16, kind="ExternalInput").ap()
    wiT_d = nc.dram_tensor("wiT", [4, 128, RT * 128], dt.bfloat16, kind="ExternalInput").ap()
    whT_d = nc.dram_tensor("whT", [4, 128, RT * 128], dt.bfloat16, kind="ExternalInput").ap()
    bias_d = nc.dram_tensor("bias", [128, RT], dt.float32, kind="ExternalInput").ap()
    h0T_d = nc.dram_tensor("h0T", [128, 4, Bs], dt.bfloat16, kind="ExternalInput").ap()
    c0T_d = nc.dram_tensor("c0T", [128, RT4, Bs], dt.float32, kind="ExternalInput").ap()
    id_d = nc.dram_tensor("ident", [128, 128], dt.bfloat16, kind="ExternalInput").ap()

    ysT_d = nc.dram_tensor("ysT", [Tn, 128, RT4 * Bs], dt.bfloat16, kind="ExternalOutput").ap()
    hfin_d = nc.dram_tensor("hfin", [128, RT4 * Bs], dt.float32, kind="ExternalOutput").ap()
    cfin_d = nc.dram_tensor("cfin", [128, RT4 * Bs], dt.float32, kind="ExternalOutput").ap()

    if use_remote:
        h_sem = nc.alloc_semaphore("h_sem")
        send_sem = nc.alloc_semaphore("send_sem")

    AF = mybir.ActivationFunctionType

    with tile.TileContext(nc) as tc:
        with (
            tc.tile_pool(name="persist", bufs=1) as persist,
            tc.tile_pool(name="xc", bufs=3) as xcp,
            tc.tile_pool(name="gpsum", bufs=3, space="PSUM") as gpsum,
            tc.tile_pool(name="spsum", bufs=2, space="PSUM") as spsum,
            tc.tile_pool(name="ew", bufs=2) as ew,
        ):
            wiT = persist.tile([128, 4, RT * 128], dt.bfloat16, tag="wiT")
            whT = persist.tile([128, 4, RT * 128], dt.bfloat16, tag="whT")
            bias = persist.tile([128, RT], dt.float32, tag="bias")
            ident = persist.tile([128, 128], dt.bfloat16, tag="ident")
            hT = persist.tile([128, 2, 4, Bs], dt.bfloat16, tag="hT")
            c_sb = persist.tile([128, RT4, Bs], dt.float32, tag="c_sb")
            xg = [
                persist.tile([128, RT, CH], dt.bfloat16, tag=f"xg{ch}", name=f"xg{ch}")
                for ch in range(NCH)
            ]

            for k in range(4):
                nc.sync.dma_start(wiT[:, k, :], wiT_d[k])
                nc.sync.dma_start(whT[:, k, :], whT_d[k])
            nc.sync.dma_start(bias[:], bias_d[:])
            nc.sync.dma_start(ident[:], id_d[:])
            nc.sync.dma_start(hT[:, 0, :, :], h0T_d[:])
            nc.sync.dma_start(c_sb[:], c0T_d[:])

            # ---- xg = x @ Wi^T + bias, bf16, resident in SBUF ----
            # One "work item" = (ch, nb, rt): 4 matmuls + 1 bias epilogue. The
            # xc chunk DMA is issued at the first item of each (ch, nb).
            xc_tiles = {}

            def emit_gemm_item(ch, nb, rt):
                off = ch * CH + nb * NPW
                if rt == 0:
                    xc = xcp.tile([128, 4, NPW], dt.bfloat16, tag="xc", name=f"xc{ch}_{nb}")
                    for k in range(4):
                        nc.sync.dma_start(xc[:, k, :], xT_d[k, :, off : off + NPW])
                    xc_tiles[(ch, nb)] = xc
                xc = xc_tiles[(ch, nb)]
                ps = gpsum.tile([128, NPW], dt.float32, tag="gps", name=f"gps{ch}_{nb}_{rt}")
                for k in range(4):
                    nc.tensor.matmul(
                        ps[:],
                        wiT[:, k, rt * 128 : (rt + 1) * 128],
                        xc[:, k, :],
                        start=(k == 0),
                        stop=(k == 3),
                    )
                dst = xg[ch][:, rt, nb * NPW : (nb + 1) * NPW]
                if rt % 2 == 0:
                    nc.scalar.activation(
                        dst, ps[:], AF.Identity, bias=bias[:, rt : rt + 1]
                    )
                else:
                    nc.vector.tensor_scalar_add(dst, ps[:], bias[:, rt : rt + 1])

            # chunks 0..PRE-1 upfront; chunks PRE.. are interleaved into the
            # recurrence steps of chunk c-PRE (PE consumes them in its idle
            # windows while the elementwise chain runs).
            PRE = 1
            for ch in range(PRE):
                for nb in range(NB):
                    for rt in range(RT):
                        emit_gemm_item(ch, nb, rt)

            TCH_steps = Tn // NCH
            items_per_step = -(-(NB * RT) // TCH_steps)  # ceil

            def emit_interleaved_gemm(t):
                ch = t // TCH_steps + PRE
                if ch >= NCH:
                    return
                pos = t % TCH_steps
                for it in range(pos * items_per_step, min((pos + 1) * items_per_step, NB * RT)):
                    emit_gemm_item(ch, it // RT, it % RT)

            # ---- recurrence ----
            for rep in range(reps):
              for t in range(Tn):
                cur, nxt = t % 2, (t + 1) % 2
                ch, tloc = t // TCH_steps, t % TCH_steps
                xg_t = xg[ch]
                ps = spsum.tile([128, RT, Bs], dt.float32, tag="sps")

                # xg injection via identity matmuls (N = RT*Bs/n_idmm <= 512)
                n_idmm = max(1, (RT * Bs) // 512)
                rt_per = RT // n_idmm
                for q in range(n_idmm):
                    nc.tensor.matmul(
                        ps[:, q * rt_per : (q + 1) * rt_per, :],
                        ident[:],
                        xg_t[:, q * rt_per : (q + 1) * rt_per, tloc * Bs : (tloc + 1) * Bs],
                        start=(q == 0),
                        stop=False,
                        skip_group_check=True,
                    )

                def wh_mms():
                    for rt in range(RT):
                        for k in range(4):
                            nc.tensor.matmul(
                                ps[:, rt, :],
                                whT[:, k, rt * 128 : (rt + 1) * 128],
                                hT[:, cur, k, :],
                                start=False,
                                stop=(rt == RT - 1 and k == 3),
                                skip_group_check=True,
                            )

                if use_remote and t >= 1:
                    with tc.tile_critical():
                        nc.tensor.wait_ge(h_sem, 6 * t)
                        wh_mms()
                else:
                    wh_mms()

                # elementwise: row-tiles [0:RT4]=i [RT4:2RT4]=f [2RT4:3RT4]=o [3RT4:RT]=g
                sig = ew.tile([128, 3 * RT4, Bs], dt.float32, tag="sig")
                tng = ew.tile([128, RT4, Bs], dt.float32, tag="tng")
                nc.scalar.activation(sig[:], ps[:, 0 : 3 * RT4, :], AF.Sigmoid)
                nc.scalar.activation(tng[:], ps[:, 3 * RT4 : RT, :], AF.Tanh)
                ig = ew.tile([128, RT4, Bs], dt.float32, tag="ig")
                fc = ew.tile([128, RT4, Bs], dt.float32, tag="fc")
                nc.vector.tensor_mul(ig[:], sig[:, 0:RT4, :], tng[:])
                nc.vector.tensor_mul(fc[:], sig[:, RT4 : 2 * RT4, :], c_sb[:])
                nc.vector.tensor_add(c_sb[:], ig[:], fc[:])
                tcn = ew.tile([128, RT4, Bs], dt.float32, tag="tc")
                nc.scalar.activation(tcn[:], c_sb[:], AF.Tanh)
                if use_remote:
                    hdst = hT[:, nxt, 0:RT4, :]
                else:
                    hdst = hT[:, nxt, :, :]
                nc.vector.tensor_mul(hdst, sig[:, 2 * RT4 : 3 * RT4, :], tcn[:])

                if use_remote:
                    with tc.tile_critical():
                        for d in (1, 2, 3):
                            nc.gpsimd.remote_dma_broadcast(
                                hT[:, nxt, d, :],
                                hT[:, nxt, 0, :],
                                remote_sem=h_sem,
                                local_sem=send_sem,
                                rdests=[(0, d)] + [None] * 7,
                            )
                        nc.gpsimd.trigger_dma(count=None)

                if rep == 0:
                    nc.sync.dma_start(ysT_d[t], hT[:, nxt, 0:RT4, :])

                if t == Tn - 1 and rep == 0:
                    hf = ew.tile([128, RT4, Bs], dt.float32, tag="hf")
                    nc.vector.tensor_mul(hf[:], sig[:, 2 * RT4 : 3 * RT4, :], tcn[:])
                    nc.sync.dma_start(hfin_d[:], hf[:])
                    nc.sync.dma_start(cfin_d[:], c_sb[:])

                if rep == 0:
                    emit_interleaved_gemm(t)

    nc.compile()
    return nc


def _rows_for(mode, rank):
    """Global gate-row indices (into 4H) for this core, in row-tile order."""
    cfg = _cfg(mode)
    RT = cfg["RT"]
    if mode == "local":
        return np.concatenate([512 * q + np.arange(512) for q in GATE_ORDER])
    else:
        return np.concatenate(
            [512 * q + 128 * rank + np.arange(128) for q in GATE_ORDER]
        )


def _prep_core(mode, c, x, h0, c0, Wi, Wh, bi, bh):
    cfg = _cfg(mode)
    Bs, RT = cfg["Bs"], cfg["RT"]
    RT4 = RT // 4
    d, rank = c // 4, c % 4
    rows = _rows_for(mode, rank)

    if mode == "local":
        bsl = slice(rank * Bs, (rank + 1) * Bs)
        hsl = np.arange(H)
        slot_slices = np.arange(4)  # hT slot j <- H-slice j
    else:
        bsl = slice(0, B)
        hsl = 128 * rank + np.arange(128)
        slot_slices = np.array([rank ^ j for j in range(4)])

    xx = x[::-1] if d == 1 else x
    xx = xx[:, bsl, :]  # [T, Bs, I]
    Tn = xx.shape[0]
    xT = np.ascontiguousarray(
        xx.astype(BF16).transpose(2, 0, 1).reshape(4, 128, Tn * Bs)
    )

    wi = Wi[rows].astype(BF16)  # [RT*128, I]
    wiT = np.ascontiguousarray(wi.T.reshape(4, 128, RT * 128))
    wh = Wh[rows].astype(BF16).T  # [H, RT*128]
    whT = np.stack(
        [wh[128 * s : 128 * s + 128] for s in slot_slices], axis=0
    )  # [4, 128, RT*128]
    bias = (bi + bh)[rows].astype(F32).reshape(RT, 128).T.copy()  # [128, RT]

    h0T = np.stack(
        [h0[bsl, 128 * s : 128 * s + 128].T.astype(BF16) for s in slot_slices], axis=1
    )  # [128, 4, Bs]
    if mode == "local":
        c0T = np.ascontiguousarray(c0[bsl].T.astype(F32).reshape(RT4, 128, Bs).transpose(1, 0, 2))
    else:
        c0T = c0[bsl, hsl.min() : hsl.min() + 128].T.astype(F32).reshape(128, 1, Bs)

    return {
        "xT": xT,
        "wiT": np.ascontiguousarray(wiT),
        "whT": np.ascontiguousarray(whT),
        "bias": np.ascontiguousarray(bias),
        "h0T": np.ascontiguousarray(h0T),
        "c0T": np.ascontiguousarray(c0T),
        "ident": np.eye(128, dtype=BF16),
    }


def kernel(x, h0_f, c0_f, h0_b, c0_b, Wi_f, Wh_f, bi_f, bh_f, Wi_b, Wh_b, bi_b, bh_b):
    mode = MODE
    cfg = _cfg(mode)
    Bs, RT = cfg["Bs"], cfg["RT"]
    RT4 = RT // 4
    x = np.asarray(x, dtype=F32)
    Tn = x.shape[0]

    if mode not in _GRAPH_CACHE:
        _GRAPH_CACHE[mode] = build_graph(mode, Tn)
    nc = _GRAPH_CACHE[mode]

    per_dir = [
        (h0_f, c0_f, Wi_f, Wh_f, bi_f, bh_f),
        (h0_b, c0_b, Wi_b, Wh_b, bi_b, bh_b),
    ]
    in_maps = []
    for c in range(8):
        h0, c0, Wi, Wh, bi, bh = [np.asarray(a, dtype=F32) for a in per_dir[c // 4]]
        in_maps.append(_prep_core(mode, c, x, h0, c0, Wi, Wh, bi, bh))

    res = bass_utils.run_bass_kernel_spmd(
        nc, in_maps, core_ids=list(range(8)), trace=TRACE
    )
    global LAST_RESULT
    LAST_RESULT = res

    out = np.zeros((Tn, B, 2 * H), dtype=F32)
    hf = np.zeros((B, H), dtype=F32)
    cf = np.zeros((B, H), dtype=F32)
    hb = np.zeros((B, H), dtype=F32)
    cb = np.zeros((B, H), dtype=F32)
    for c in range(8):
        d, rank = c // 4, c % 4
        r = res.results[c]
        ys = np.asarray(r["ysT"]).astype(F32).reshape(Tn, 128, RT4, Bs)
        hfin = np.asarray(r["hfin"]).astype(F32).reshape(128, RT4, Bs)
        cfin = np.asarray(r["cfin"]).astype(F32).reshape(128, RT4, Bs)
        ys = ys.transpose(0, 3, 2, 1).reshape(Tn, Bs, RT4 * 128)  # [T, Bs, dims]
        hfin = hfin.transpose(2, 1, 0).reshape(Bs, RT4 * 128)
        cfin = cfin.transpose(2, 1, 0).reshape(Bs, RT4 * 128)
        if d == 1:
            ys = ys[::-1]
        if mode == "local":
            bsl = slice(rank * Bs, (rank + 1) * Bs)
            dsl = slice(0, H)
        else:
            bsl = slice(0, B)
            dsl = slice(rank * 128, rank * 128 + 128)
        out[:, bsl, (d * H + dsl.start) : (d * H + dsl.stop)] = ys
        (hf if d == 0 else hb)[bsl, dsl] = hfin
        (cf if d == 0 else cb)[bsl, dsl] = cfin

    return out, hf, cf, hb, cb


if __name__ == "__main__":
    rng = np.random.default_rng(0)
    ins = {
        "x": rng.standard_normal((T, B, I), dtype=F32),
        "h0_f": np.zeros((B, H), F32),
        "c0_f": np.zeros((B, H), F32),
        "h0_b": np.zeros((B, H), F32),
        "c0_b": np.zeros((B, H), F32),
    }
    for dd in ("f", "b"):
        ins[f"Wi_{dd}"] = (rng.standard_normal((4 * H, I), dtype=F32) / np.sqrt(I)).astype(F32)
        ins[f"Wh_{dd}"] = (rng.standard_normal((4 * H, H), dtype=F32) / np.sqrt(H)).astype(F32)
        ins[f"bi_{dd}"] = rng.standard_normal(4 * H, dtype=F32) / np.sqrt(H)
        ins[f"bh_{dd}"] = rng.standard_normal(4 * H, dtype=F32) / np.sqrt(H)
    outs = kernel(**ins)
    print([o.shape for o in outs])
